# revision 26
# baseline (speedup 1.0000x reference)
"""AlignmentEncoder forward on 8 Trainium2 NeuronCores (data-parallel over batch).

v2: restructured from the 65-71us baseline using trace evidence:
  - fp8 (e4m3) kw1 + wcat weights (x64 host scale, /64 folded into kw2 / z)
    -> halves the two biggest DMA transfers; matmuls run mixed fp8xbf16.
  - all small inputs packed host-side into 3 blobs (bf16 / f32 / fp8) ->
    ~17 input DMA instructions shrink to 8, split across BOTH HWDGE rings
    (SP + ACT) so the ~650ns-per-DMA issue cost stops serializing the
    middle of the kernel.
  - padding columns, pmask broadcast, ones vectors: pre-packed host-side
    (no device memsets / broadcast DMAs).
  - prior: host adds 1e-8 and casts bf16 (0.5MB instead of 1MB + memset +
    8 SWDGE accum DMAs).
  - epilogue: bf16 intermediates, ops spread over ACT/DVE/GpSimd, emission
    software-pipelined (exp of chunk c+1 issued before ln of chunk c) so
    the per-chunk cadence is engine-bound, not dependency-bound.
  - outputs: single bf16 [1000, 512] tensor (logp | attn), 8 chunk DMAs,
    host casts to f32.
Math (per batch b, one core):
  x[t,s] = 2*TEMP*qk - TEMP*k2  (q2 term dropped: row-constant cancels)
  out_logp = ln(exp(x)*prior_eps * 1/sum_s exp(x)) = x - lse + log(prior+1e-8)
  out_attn = wm / sum_s wm,  wm = exp(x)*prior_eps*pmask
"""

import sys
from contextlib import ExitStack

sys.path.insert(0, "/opt/trn_rl_repo")

import numpy as np
import ml_dtypes

import bass_rust
from bass_rust import add_dep_helper
import concourse.bass as bass
import concourse.mybir as mybir
import concourse.tile as tile

BF16 = ml_dtypes.bfloat16
FP8 = ml_dtypes.float8_e3m4   # wcat (no DoubleRow)
FP8D = ml_dtypes.float8_e4m3  # kw1 + keys (DoubleRow pairs)
F32 = np.float32

B, MEL, TXT, ATT, T1, T2 = 8, 80, 512, 80, 1000, 256
TEMP = 0.0005
N_CORES = 8
WS = 64.0   # kw1 fp8 scale
WSW = 32.0  # wcat fp8 scale (e3m4 max ~15.5; keep 8-sigma headroom)

_MAX_WAITS = 1  # this walrus build rejects multi-wait instructions

# blobZ (bf16, early: gates proj + q chain) column offsets
ZB = 0          # z/64              [128, 8]
ONC = 8         # ones column       [128, 1]
ONR = 9         # ones rows         [128, 128] (row 0 used as [1,128] lhsT)
QW1 = 137       # qw1 (k*160+c)     [80, 480]
QW2A = 617      # qw2a              [128, 80]
QW2B = 697      # qw2b              [32, 80]
QW3 = 777       # qw3               [80, 80]
NZ = 857
# blobM (bf16, late: gates conv2 + epilogue) column offsets
PMK = 0         # pmask broadcast, duplicated for chunk pairs [128, 512]
KW2 = 512       # kw2/64 (co*80+a)  [128, 640]
NM = 1152

# blobF (f32) column offsets
FBV = 0         # bvec              [128, 5]
FKB1 = 5        # kb1*64            [128, 8]
FKB2 = 13       # kb2               [80, 1]
FQB1A = 14      # qb1[0:128]        [128, 1]
FQB1B = 15      # qb1[128:160]      [32, 1]
FQB2 = 16       # qb2               [80, 1]
FQB3 = 17       # qb3               [80, 1]
NF = 18

# blobA (bf16): keys 4x258 then queries 1002 (zero-padded borders)
QOFF = 4 * 258  # 1032
NA = QOFF + T1 + 2  # 2034


def _split_excess_waits(nc):
    """Move excess sem waits onto single-wait NoOps (walrus limit)."""
    spread = [
        mybir.EngineType.SP,
        mybir.EngineType.Activation,
        mybir.EngineType.DVE,
        mybir.EngineType.PE,
        mybir.EngineType.Pool,
    ]
    uid = 0
    for blk in nc.m.functions[0].blocks:
        insts = list(blk.instructions)
        out = []
        changed = False
        for inst in insts:
            si = inst.sync_info
            waits = list(si.on_wait) if si is not None and si.on_wait else []
            if len(waits) > _MAX_WAITS:
                si.on_wait = waits[-_MAX_WAITS:]
                extra = waits[:-_MAX_WAITS]
                is_drain = isinstance(inst, mybir.InstDrain)
                for j in range(0, len(extra), _MAX_WAITS):
                    nop = mybir.InstNoOp(name=f"I-waitsplit-{uid}", ins=[], outs=[])
                    nop.engine = (
                        spread[(j // _MAX_WAITS) % len(spread)]
                        if is_drain
                        else inst.engine
                    )
                    uid += 1
                    nop.bass_nofuse = True
                    nop.sync_info = bass_rust.SyncInfo(
                        on_wait=extra[j : j + _MAX_WAITS], on_update=[]
                    )
                    out.append(nop)
                changed = True
            out.append(inst)
        if changed:
            blk.instructions = out


class _TC(tile.TileContext):
    pass


def _build_nc(split_waits=True):
    f32 = mybir.dt.float32
    bf16 = mybir.dt.bfloat16
    fp8 = mybir.dt.float8e3
    fp8d = mybir.dt.float8e4
    AF = mybir.ActivationFunctionType
    AX = mybir.AxisListType

    nc = bass.Bass("TRN2", target_bir_lowering=False, debug=False, num_devices=N_CORES)
    dp = nc.declare_dram_parameter
    wz_p = dp("wz", [128, 8, 592], fp8, isOutput=False)
    kw1a_p = dp("kw1a", [128, 3, 3, 2, 2, 128], fp8d, isOutput=False)
    kw1b_p = dp("kw1b", [128, 3, 3, 2, 2, 128], fp8d, isOutput=False)
    kw1c_p = dp("kw1c", [128, 2, 3, 2, 2, 128], fp8d, isOutput=False)
    zb_p = dp("zb", [128, 8], bf16, isOutput=False)
    keysA_p = dp("keysA", [128, 1032], bf16, isOutput=False)
    queriesA_p = dp("queriesA", [128, T1 + 2], bf16, isOutput=False)
    blobZ_p = dp("blobZ", [128, NZ], bf16, isOutput=False)
    blobM_p = dp("blobM", [128, NM], bf16, isOutput=False)
    blobF_p = dp("blobF", [128, NF], f32, isOutput=False)
    pb_p = dp("pb", [128, 8, T2], bf16, isOutput=False)
    out_p = dp("out", [T1, 2 * T2], bf16, isOutput=True)

    with _TC(nc) as tc, ExitStack() as ctx:
        const = ctx.enter_context(tc.tile_pool(name="const", bufs=1))
        work = ctx.enter_context(tc.tile_pool(name="work", bufs=4))
        small = ctx.enter_context(tc.tile_pool(name="small", bufs=8))

        # ---- warmup tiles (memset so sim sees initialized data)
        warm_w = const.tile([128, 8], bf16, tag="warmw")
        nc.vector.memset(warm_w, 0.0)
        warm_rhs = const.tile([128, 512], bf16, tag="warmrhs")
        nc.vector.memset(warm_rhs, 0.0)
        # fp8 conditioned keys for DoubleRow conv1; 272-col stride (16-mult)
        keys_f8 = const.tile([128, 4, 272], fp8d, tag="keysf8")
        nc.vector.memset(keys_f8[:, :, 0:1], 0.0)
        nc.vector.memset(keys_f8[:, :, 257:258], 0.0)

        # ---- input DMAs.  SP ring: fp8 weights; ACT ring: blobs + prior.
        # Critical set (gates compute start): wz, blobZ, blobA, blobF,
        # blobM, kw1a.  kw1b and pb are bandwidth-gated behind it so the
        # critical set gets the full ~360GB/s (total input is HBM-bound).
        # Q1 (SP ring) carries the strict critical chain in need-order:
        # wz (proj) -> keysA (conditioning) -> kw1a/b/c (conv1 blocks).
        # Q10 (ACT ring) carries the small/late tensors.
        wz_sb = const.tile([128, 8, 592], fp8, tag="wz")
        nc.sync.dma_start(out=wz_sb, in_=wz_p[:])
        keysA = const.tile([128, 1032], bf16, tag="keysA")
        nc.sync.dma_start(out=keysA, in_=keysA_p[:])
        kw1_sl = []
        for nm, p in (("kw1a", kw1a_p), ("kw1b", kw1b_p), ("kw1c", kw1c_p)):
            t = const.tile(list(p.shape), fp8d, tag=nm, name=nm)
            nc.sync.dma_start(out=t, in_=p[:])
            kw1_sl.append(t)
        zb_sb = const.tile([128, 8], bf16, tag="zb")
        nc.scalar.dma_start(out=zb_sb, in_=zb_p[:])
        blobF = const.tile([128, NF], f32, tag="blobF")
        nc.scalar.dma_start(out=blobF, in_=blobF_p[:])
        queriesA = const.tile([128, T1 + 2], bf16, tag="queriesA")
        nc.scalar.dma_start(out=queriesA, in_=queriesA_p[:])
        blobZ = const.tile([128, NZ], bf16, tag="blobZ")
        nc.scalar.dma_start(out=blobZ, in_=blobZ_p[:])
        blobM = const.tile([128, NM], bf16, tag="blobM")
        nc.scalar.dma_start(out=blobM, in_=blobM_p[:])
        pb_sb = const.tile([128, 8, T2], bf16, tag="pb")
        nc.scalar.dma_start(out=pb_sb, in_=pb_p[:])

        def kw1w(co, m, k):
            g = min(co // 3, 2)
            return kw1_sl[g][:, co - 3 * g, k, m]

        # persistent compute tiles
        keys1_bf = const.tile([128, 8, T2], bf16, tag="keys1")
        q1a_bf = const.tile([128, T1], bf16, tag="q1a")
        q1b_bf = const.tile([32, T1], bf16, tag="q1b")
        q2_bf = const.tile([MEL, T1], bf16, tag="q2bf")
        Qp = const.tile([MEL, 1024], bf16, tag="Qp")
        nc.vector.memset(Qp[:, T1:1024], 0.0)
        kenc_sb = const.tile([ATT, T2], bf16, tag="kenc")
        sk_sb = const.tile([ATT, T2], bf16, tag="sk")
        k2neg_sb = const.tile([1, T2], bf16, tag="k2neg")
        proj_sb = const.tile([128, 5], f32, tag="proj_sb")

        # ---- warmup + proj on PE
        with tc.tile_pool(name="psA", bufs=1, space="PSUM") as psA:
            warm_ps = psA.tile([8, 512], f32, tag="warm")
            for _ in range(8):
                nc.tensor.matmul(
                    warm_ps, lhsT=warm_w, rhs=warm_rhs, start=True, stop=True
                )
            proj_ps = psA.tile([128, 5], f32, tag="proj")
            for cb in range(5):
                mw = 128 if cb < 4 else 80
                for jc in range(8):
                    nc.tensor.matmul(
                        proj_ps[0:mw, cb : cb + 1],
                        lhsT=wz_sb[:, jc, cb * 128 : cb * 128 + mw],
                        rhs=zb_sb[:, jc : jc + 1],
                        start=(jc == 0),
                        stop=(jc == 7),
                    )
                # the tiny proj matmuls don't register as PE activity; keep
                # the HAM clock gate open through proj + conditioning
                nc.tensor.matmul(
                    warm_ps, lhsT=warm_w, rhs=warm_rhs, start=True, stop=True
                )
            for _ in range(2):  # PE busy while DVE does proj-add + cond
                nc.tensor.matmul(
                    warm_ps, lhsT=warm_w, rhs=warm_rhs, start=True, stop=True
                )
            nc.vector.tensor_add(
                proj_sb[:, 0:4], proj_ps[:, 0:4], blobF[:, FBV : FBV + 4]
            )
            nc.vector.tensor_add(
                proj_sb[0:MEL, 4:5], proj_ps[0:MEL, 4:5],
                blobF[0:MEL, FBV + 4 : FBV + 5],
            )

        # ---- conditioning.  Queries in place (DVE, first: unblocks q1);
        # keys into keys_f8 (fp8 for DoubleRow): ci0/ci1 DVE, ci2/ci3 ACT
        # (Identity with per-partition bias) to halve the wall time.
        nc.vector.tensor_scalar_add(
            queriesA[0:MEL, 1 : 1 + T1],
            queriesA[0:MEL, 1 : 1 + T1],
            proj_sb[0:MEL, 4:5],
        )
        for ci in range(4):
            nc.vector.tensor_scalar_add(
                keys_f8[:, ci, 1:257],
                keysA[:, ci * 258 + 1 : ci * 258 + 257],
                proj_sb[:, ci : ci + 1],
            )

        # ---- conv chains (keys chain prioritized; q blocks fill gaps)
        with (
            tc.tile_pool(name="psB", bufs=1, space="PSUM") as psB,
            tc.tile_pool(name="psC", bufs=1, space="PSUM") as psC,
        ):
            def q1_block(t0):
                q1a_ps = psB.tile([128, 500], f32, tag="q1a")
                q1b_ps = psB.tile([32, 500], f32, tag="q1b")
                for k in range(3):
                    nc.tensor.matmul(
                        q1a_ps,
                        lhsT=blobZ[0:MEL, QW1 + k * 160 : QW1 + k * 160 + 128],
                        rhs=queriesA[0:MEL, t0 + k : t0 + k + 500],
                        start=(k == 0),
                        stop=(k == 2),
                    )
                for k in range(3):
                    nc.tensor.matmul(
                        q1b_ps,
                        lhsT=blobZ[0:MEL, QW1 + k * 160 + 128 : QW1 + k * 160 + 160],
                        rhs=queriesA[0:MEL, t0 + k : t0 + k + 500],
                        start=(k == 0),
                        stop=(k == 2),
                    )
                nc.vector.tensor_scalar(
                    q1a_bf[:, t0 : t0 + 500], q1a_ps,
                    blobF[:, FQB1A : FQB1A + 1], 0.0,
                    op0=mybir.AluOpType.add, op1=mybir.AluOpType.max,
                )
                nc.scalar.activation(
                    q1b_bf[:, t0 : t0 + 500], q1b_ps, AF.Relu,
                    bias=blobF[0:32, FQB1B : FQB1B + 1],
                )

            def q2_block(t0):
                q2_ps = psB.tile([MEL, 500], f32, tag="q2")
                nc.tensor.matmul(
                    q2_ps, lhsT=blobZ[:, QW2A : QW2A + MEL],
                    rhs=q1a_bf[:, t0 : t0 + 500], start=True, stop=False,
                )
                nc.tensor.matmul(
                    q2_ps, lhsT=blobZ[0:32, QW2B : QW2B + MEL],
                    rhs=q1b_bf[:, t0 : t0 + 500], start=False, stop=True,
                )
                nc.scalar.activation(
                    q2_bf[:, t0 : t0 + 500], q2_ps, AF.Relu,
                    bias=blobF[0:MEL, FQB2 : FQB2 + 1],
                )

            def q3_block(t0):
                q3_ps = psB.tile([MEL, 500], f32, tag="q3")
                nc.tensor.matmul(
                    q3_ps, lhsT=blobZ[0:MEL, QW3 : QW3 + MEL],
                    rhs=q2_bf[:, t0 : t0 + 500], start=True, stop=True,
                )
                nc.vector.tensor_scalar(
                    Qp[:, t0 : t0 + 500], q3_ps,
                    blobF[0:MEL, FQB3 : FQB3 + 1], 2.0 * TEMP,
                    op0=mybir.AluOpType.add, op1=mybir.AluOpType.mult,
                )

            k_ps = psC.tile([ATT, T2], f32, tag="kps", bufs=1)

            def conv1_block(co):
                c1_ps = psC.tile([128, T2], f32, tag="c1", bufs=3)
                n = 0
                for m in range(2):
                    for k in range(3):
                        v = keys_f8[:, 2 * m, k : k + T2]
                        rhs = bass.AP(
                            tensor=v.tensor, offset=v.offset,
                            ap=[list(v.ap[0]), [272, 2], [1, T2]],
                        )
                        nc.tensor.matmul(
                            c1_ps,
                            lhsT=kw1w(co, m, k),
                            rhs=rhs,
                            start=(n == 0),
                            stop=(n == 5),
                            perf_mode=mybir.MatmulPerfMode.DoubleRow,
                        )
                        n += 1
                if co % 2 == 0:
                    nc.scalar.activation(
                        keys1_bf[:, co, :], c1_ps, AF.Relu,
                        bias=blobF[:, FKB1 + co : FKB1 + co + 1],
                    )
                else:
                    nc.vector.tensor_scalar(
                        keys1_bf[:, co, :], c1_ps,
                        blobF[:, FKB1 + co : FKB1 + co + 1], 0.0,
                        op0=mybir.AluOpType.add, op1=mybir.AluOpType.max,
                    )
                if co >= 3:
                    conv2_block(co - 3)

            def conv2_block(co):
                nc.tensor.matmul(
                    k_ps,
                    lhsT=blobM[:, KW2 + co * MEL : KW2 + (co + 1) * MEL],
                    rhs=keys1_bf[:, co, :],
                    start=(co == 0),
                    stop=(co == 7),
                )

            q1_block(0)
            # keep the PE clock hot across the cond->conv1 data-wait window
            warm2_ps = psB.tile([8, 512], f32, tag="q3", name="warm2_ps")
            for _ in range(3):
                nc.tensor.matmul(
                    warm2_ps, lhsT=warm_w, rhs=warm_rhs, start=True, stop=True
                )
            conv1_block(0)
            conv1_block(1)
            q1_block(500)
            conv1_block(2)
            q2_block(0)
            conv1_block(3)
            q2_block(500)
            conv1_block(4)
            conv1_block(5)
            q3_block(0)
            conv1_block(6)
            conv1_block(7)
            conv2_block(5)
            conv2_block(6)
            conv2_block(7)
            q3_block(500)

            nc.vector.tensor_scalar_add(
                kenc_sb, k_ps, blobF[0:ATT, FKB2 : FKB2 + 1]
            )
            nc.scalar.activation(
                sk_sb, k_ps, AF.Square, bias=blobF[0:ATT, FKB2 : FKB2 + 1]
            )

        # ---- attention scores + double softmax, chunk-pipelined
        with (
            tc.tile_pool(name="psD", bufs=1, space="PSUM") as psD,
        ):
            k2_ps = psD.tile([1, T2], f32, tag="k2", bufs=1)
            nc.tensor.matmul(
                k2_ps, lhsT=blobZ[0:ATT, ONC : ONC + 1], rhs=sk_sb,
                start=True, stop=True,
            )
            nc.scalar.activation(k2neg_sb, k2_ps, AF.Copy, scale=-TEMP)

            # chunk PAIRS share one full PSUM bank [128, 512]: the first qk
            # (start=True) clears the bank, the second (start=False) lands on
            # clean has_written bits and overwrites; both rank-1 closes then
            # accumulate.  Halves per-op overhead on ACT/DVE.
            a_tiles = {}
            for p in range(4):
                a_ps = psD.tile([128, 2 * T2], f32, tag="attn", bufs=3)
                for j in range(2):
                    nc.tensor.matmul(
                        a_ps[:, j * T2 : (j + 1) * T2],
                        lhsT=Qp[:, (2 * p + j) * 128 : (2 * p + j) * 128 + 128],
                        rhs=kenc_sb,
                        start=(j == 0),
                        stop=False,
                    )
                for j in range(2):
                    nc.tensor.matmul(
                        a_ps[:, j * T2 : (j + 1) * T2],
                        lhsT=blobZ[0:1, ONR : ONR + 128],
                        rhs=k2neg_sb,
                        start=False,
                        stop=(j == 1),
                    )
                a_tiles[p] = a_ps

            e_t, s_t, r_t, ep_t = {}, {}, {}, {}
            rows_of = lambda c: 128 if c < 7 else T1 - 7 * 128

            def ep_exp(p):
                e2 = work.tile([128, 2 * T2], bf16, tag="e", bufs=4, name="e2")
                nc.scalar.activation(e2, a_tiles[p], AF.Exp)
                e_t[p] = e2

            def ep_mid(p):
                e2 = e_t[p]
                s2t = small.tile([128, 2], f32, tag="s", name="s2t")
                ev = bass.AP(
                    tensor=e2.tensor, offset=e2.offset,
                    ap=[list(e2.ap[0]), [T2, 2], [1, T2]],
                )
                nc.vector.reduce_sum(s2t, ev, axis=AX.X)
                r = small.tile([128, 2], f32, tag="r", name="r")
                nc.vector.reciprocal(r, s2t)
                ep2 = work.tile([128, 2 * T2], bf16, tag="ep", bufs=4, name="ep2")
                nc.gpsimd.tensor_mul(ep2, e2, pb_sb[:, 2 * p : 2 * p + 2, :])
                r_t[p], ep_t[p] = r, ep2

            def ep_tail(p):
                ep2, r = ep_t[p], r_t[p]
                # ln per chunk (scale is per-partition, so one op per chunk)
                osb2 = {}
                for j in range(2):
                    rows = rows_of(2 * p + j)
                    ob = work.tile(
                        [128, 2 * T2], bf16, tag="osb", bufs=4, name="osb"
                    )
                    nc.scalar.activation(
                        ob[0:rows, 0:T2],
                        ep2[0:rows, j * T2 : (j + 1) * T2],
                        AF.Ln, scale=r[0:rows, j : j + 1],
                    )
                    osb2[j] = ob
                wm2 = work.tile([128, 2 * T2], bf16, tag="wm", bufs=4, name="wm2")
                nc.vector.tensor_mul(wm2, ep2, blobM[:, PMK : PMK + 2 * T2])
                q2t = small.tile([128, 2], f32, tag="s2", name="q2t")
                wv = bass.AP(
                    tensor=wm2.tensor, offset=wm2.offset,
                    ap=[list(wm2.ap[0]), [T2, 2], [1, T2]],
                )
                nc.vector.reduce_sum(q2t, wv, axis=AX.X)
                r2 = small.tile([128, 2], f32, tag="r2", name="r2")
                nc.vector.reciprocal(r2, q2t)
                for j in range(2):
                    rows = rows_of(2 * p + j)
                    nc.vector.tensor_scalar_mul(
                        osb2[j][0:rows, T2 : 2 * T2],
                        wm2[0:rows, j * T2 : (j + 1) * T2],
                        r2[0:rows, j : j + 1],
                    )
                    nc.sync.dma_start(
                        out=out_p[(2 * p + j) * 128 : (2 * p + j) * 128 + rows, :],
                        in_=osb2[j][0:rows, :],
                    )

            ep_exp(0)
            ep_exp(1)
            for p in range(4):
                if p < 2:
                    ep_exp(p + 2)
                ep_mid(p)
                ep_tail(p)

    if split_waits:
        _split_excess_waits(nc)
    return nc


_NC_CACHE = {}


def _get_nc():
    if "nc" not in _NC_CACHE:
        _NC_CACHE["nc"] = _build_nc()
    return _NC_CACHE["nc"]


def _prep_in_maps(inputs):
    q = np.asarray(inputs["queries"], F32)
    k = np.asarray(inputs["keys"], F32)
    mask = np.asarray(inputs["mask"])
    prior = np.asarray(inputs["attn_prior"], F32)
    spk = np.asarray(inputs["speaker_embed"], F32)
    emo = np.asarray(inputs["emotion_embed"], F32)

    kw1 = np.asarray(inputs["kw1"], F32)
    kb1 = np.asarray(inputs["kb1"], F32)
    kw2 = np.asarray(inputs["kw2"], F32)
    kb2 = np.asarray(inputs["kb2"], F32)
    qw1 = np.asarray(inputs["qw1"], F32)
    qb1 = np.asarray(inputs["qb1"], F32)
    qw2 = np.asarray(inputs["qw2"], F32)
    qb2 = np.asarray(inputs["qb2"], F32)
    qw3 = np.asarray(inputs["qw3"], F32)
    qb3 = np.asarray(inputs["qb3"], F32)
    spk_kw = np.asarray(inputs["spk_kw"], F32)
    spk_kb = np.asarray(inputs["spk_kb"], F32)
    spk_qw = np.asarray(inputs["spk_qw"], F32)
    spk_qb = np.asarray(inputs["spk_qb"], F32)
    emo_kw = np.asarray(inputs["emo_kw"], F32)
    emo_kb = np.asarray(inputs["emo_kb"], F32)
    emo_qw = np.asarray(inputs["emo_qw"], F32)
    emo_qb = np.asarray(inputs["emo_qb"], F32)

    # wz: concat speaker/emotion proj weights, x64, fp8  [128, 8, 592]
    wcat = np.concatenate(
        [
            np.concatenate([spk_kw, emo_kw], axis=1),
            np.concatenate([spk_qw, emo_qw], axis=1),
        ],
        axis=0,
    )  # (592, 1024)
    wz = np.ascontiguousarray(
        (wcat.T.reshape(8, 128, 592).transpose(1, 0, 2)) * WSW
    ).astype(FP8)

    # kw1 x64 fp8, (j, co, ci, k, c); split over co
    # pairs layout [j, co_hi, k, m, sub, c] (sub = paired ci blocks)
    kw1_t = (
        kw1.reshape(8, 128, 2, 2, 128, 3).transpose(4, 0, 5, 2, 3, 1) * WS
    ).astype(FP8D)
    kw1a = np.ascontiguousarray(kw1_t[:, 0:3])
    kw1b = np.ascontiguousarray(kw1_t[:, 3:6])
    kw1c = np.ascontiguousarray(kw1_t[:, 6:8])

    k_r = k.reshape(B, 4, 128, T2)
    # blobZ (bf16, early) / blobM (bf16, late)
    blobZ = np.zeros((128, NZ), BF16)
    blobM = np.zeros((128, NM), BF16)
    blobZ[:, ONC] = 1.0
    blobZ[:, ONR : ONR + 128] = 1.0
    for kk in range(3):
        # lhsT layout: [j(80 partitions), c(160 cols)] = qw1[c, j, kk].T
        blobZ[0:MEL, QW1 + kk * 160 : QW1 + (kk + 1) * 160] = np.ascontiguousarray(
            qw1[:, :, kk]
        ).T.astype(BF16)
    qw2t = qw2[:, :, 0].T  # (160, 80): [in-ch j, out-ch]
    blobZ[:, QW2A : QW2A + MEL] = qw2t[0:128].astype(BF16)
    blobZ[0:32, QW2B : QW2B + MEL] = qw2t[128:160].astype(BF16)
    blobZ[0:MEL, QW3 : QW3 + MEL] = qw3[:, :, 0].T.astype(BF16)
    pmask_row = (1.0 - mask[:, :, 0].astype(F32))  # (B, T2)
    kw2_t = (kw2[:, :, 0].T.reshape(8, 128, ATT).transpose(1, 0, 2) / WS).astype(BF16)
    blobM[:, KW2 : KW2 + 640] = kw2_t.reshape(128, 640)

    # blobF (f32)
    blobF = np.zeros((128, NF), F32)
    bcat = np.zeros(640, F32)
    bcat[0:TXT] = spk_kb + emo_kb
    bcat[TXT : TXT + MEL] = spk_qb + emo_qb
    blobF[:, FBV : FBV + 5] = bcat.reshape(5, 128).T
    blobF[:, FKB1 : FKB1 + 8] = kb1.reshape(8, 128).T * WS
    blobF[0:ATT, FKB2] = kb2
    blobF[:, FQB1A] = qb1[0:128]
    blobF[0:32, FQB1B] = qb1[128:160]
    blobF[0:MEL, FQB2] = qb2
    blobF[0:MEL, FQB3] = qb3

    z_all = np.concatenate([spk, emo], axis=1)  # (B, 1024)

    in_maps = []
    for b in range(B):
        m = {
            "wz": wz,
            "kw1a": kw1a,
            "kw1b": kw1b,
            "kw1c": kw1c,
        }
        kA = np.zeros((128, 1032), BF16)
        for ci in range(4):
            kA[:, ci * 258 + 1 : ci * 258 + 257] = k_r[b, ci].astype(BF16)
        m["keysA"] = kA
        qA = np.zeros((128, T1 + 2), BF16)
        qA[0:MEL, 1 : 1 + T1] = q[b].astype(BF16)
        m["queriesA"] = qA
        m["zb"] = np.ascontiguousarray(
            (z_all[b].reshape(8, 128).T / WSW)
        ).astype(BF16)
        m["blobZ"] = blobZ
        bM = blobM.copy()
        bM[:, PMK : PMK + T2] = np.broadcast_to(
            pmask_row[b].astype(BF16), (128, T2)
        )
        bM[:, PMK + T2 : PMK + 2 * T2] = bM[:, PMK : PMK + T2]
        m["blobM"] = bM
        m["blobF"] = blobF
        pbt = np.zeros((128, 8, T2), BF16)
        pr = prior[b] + 1e-8  # (1000, 256)
        pr_pad = np.zeros((1024, T2), F32)
        pr_pad[0:T1] = pr
        pbt[:, :, :] = pr_pad.reshape(8, 128, T2).transpose(1, 0, 2).astype(BF16)
        m["pb"] = pbt
        in_maps.append(m)
    return in_maps


def _unpack(results):
    attn = np.stack(
        [
            np.asarray(results[i]["out"][:, T2 : 2 * T2], F32)
            for i in range(N_CORES)
        ]
    )
    logp = np.stack(
        [np.asarray(results[i]["out"][:, 0:T2], F32) for i in range(N_CORES)]
    )
    return attn[:, None], logp[:, None]


def kernel(**inputs):
    from concourse.bass_utils import run_bass_kernel_spmd

    nc = _get_nc()
    in_maps = _prep_in_maps(inputs)
    res = run_bass_kernel_spmd(nc, in_maps, core_ids=list(range(N_CORES)))
    return _unpack(res.results)


# revision 29
# speedup vs baseline: 1.0446x; 1.0446x over previous
"""AlignmentEncoder forward on 8 Trainium2 NeuronCores (data-parallel over batch).

v2: restructured from the 65-71us baseline using trace evidence:
  - fp8 (e4m3) kw1 + wcat weights (x64 host scale, /64 folded into kw2 / z)
    -> halves the two biggest DMA transfers; matmuls run mixed fp8xbf16.
  - all small inputs packed host-side into 3 blobs (bf16 / f32 / fp8) ->
    ~17 input DMA instructions shrink to 8, split across BOTH HWDGE rings
    (SP + ACT) so the ~650ns-per-DMA issue cost stops serializing the
    middle of the kernel.
  - padding columns, pmask broadcast, ones vectors: pre-packed host-side
    (no device memsets / broadcast DMAs).
  - prior: host adds 1e-8 and casts bf16 (0.5MB instead of 1MB + memset +
    8 SWDGE accum DMAs).
  - epilogue: bf16 intermediates, ops spread over ACT/DVE/GpSimd, emission
    software-pipelined (exp of chunk c+1 issued before ln of chunk c) so
    the per-chunk cadence is engine-bound, not dependency-bound.
  - outputs: single bf16 [1000, 512] tensor (logp | attn), 8 chunk DMAs,
    host casts to f32.
Math (per batch b, one core):
  x[t,s] = 2*TEMP*qk - TEMP*k2  (q2 term dropped: row-constant cancels)
  out_logp = ln(exp(x)*prior_eps * 1/sum_s exp(x)) = x - lse + log(prior+1e-8)
  out_attn = wm / sum_s wm,  wm = exp(x)*prior_eps*pmask
"""

import sys
from contextlib import ExitStack

sys.path.insert(0, "/opt/trn_rl_repo")

import numpy as np
import ml_dtypes

import bass_rust
from bass_rust import add_dep_helper
import concourse.bass as bass
import concourse.mybir as mybir
import concourse.tile as tile

BF16 = ml_dtypes.bfloat16
FP8 = ml_dtypes.float8_e3m4   # wcat (no DoubleRow)
FP8D = ml_dtypes.float8_e4m3  # kw1 + keys (DoubleRow pairs)
F32 = np.float32

B, MEL, TXT, ATT, T1, T2 = 8, 80, 512, 80, 1000, 256
TEMP = 0.0005
N_CORES = 8
WS = 64.0   # kw1 fp8 scale
WSW = 32.0  # wcat fp8 scale (e3m4 max ~15.5; keep 8-sigma headroom)

_MAX_WAITS = 1  # this walrus build rejects multi-wait instructions

# blobZ (bf16, early: gates proj + q chain) column offsets
ZB = 0          # z/64              [128, 8]
ONC = 8         # ones column       [128, 1]
ONR = 9         # ones rows         [128, 128] (row 0 used as [1,128] lhsT)
QW1 = 137       # qw1 (k*160+c)     [80, 480]
QW2A = 617      # qw2a              [128, 80]
QW2B = 697      # qw2b              [32, 80]
QW3 = 777       # qw3               [80, 80]
NZ = 857
# blobM (bf16, late: gates conv2 + epilogue) column offsets
PMK = 0         # pmask broadcast   [128, 256]
KW2 = 256       # kw2/64 (co*80+a)  [128, 640]
NM = 896

# blobF (f32) column offsets
FBV = 0         # bvec              [128, 5]
FKB1 = 5        # kb1*64            [128, 8]
FKB2 = 13       # kb2               [80, 1]
FQB1A = 14      # qb1[0:128]        [128, 1]
FQB1B = 15      # qb1[128:160]      [32, 1]
FQB2 = 16       # qb2               [80, 1]
FQB3 = 17       # qb3               [80, 1]
NF = 18

# blobA (bf16): keys 4x258 then queries 1002 (zero-padded borders)
QOFF = 4 * 258  # 1032
NA = QOFF + T1 + 2  # 2034


def _split_excess_waits(nc):
    """Move excess sem waits onto single-wait NoOps (walrus limit)."""
    spread = [
        mybir.EngineType.SP,
        mybir.EngineType.Activation,
        mybir.EngineType.DVE,
        mybir.EngineType.PE,
        mybir.EngineType.Pool,
    ]
    uid = 0
    for blk in nc.m.functions[0].blocks:
        insts = list(blk.instructions)
        out = []
        changed = False
        for inst in insts:
            si = inst.sync_info
            waits = list(si.on_wait) if si is not None and si.on_wait else []
            if len(waits) > _MAX_WAITS:
                si.on_wait = waits[-_MAX_WAITS:]
                extra = waits[:-_MAX_WAITS]
                is_drain = isinstance(inst, mybir.InstDrain)
                for j in range(0, len(extra), _MAX_WAITS):
                    nop = mybir.InstNoOp(name=f"I-waitsplit-{uid}", ins=[], outs=[])
                    nop.engine = (
                        spread[(j // _MAX_WAITS) % len(spread)]
                        if is_drain
                        else inst.engine
                    )
                    uid += 1
                    nop.bass_nofuse = True
                    nop.sync_info = bass_rust.SyncInfo(
                        on_wait=extra[j : j + _MAX_WAITS], on_update=[]
                    )
                    out.append(nop)
                changed = True
            out.append(inst)
        if changed:
            blk.instructions = out


class _TC(tile.TileContext):
    pass


def _build_nc(split_waits=True):
    f32 = mybir.dt.float32
    bf16 = mybir.dt.bfloat16
    fp8 = mybir.dt.float8e3
    fp8d = mybir.dt.float8e4
    AF = mybir.ActivationFunctionType
    AX = mybir.AxisListType

    nc = bass.Bass("TRN2", target_bir_lowering=False, debug=False, num_devices=N_CORES)
    dp = nc.declare_dram_parameter
    wz_p = dp("wz", [128, 8, 592], fp8, isOutput=False)
    kw1a_p = dp("kw1a", [128, 3, 3, 2, 2, 128], fp8d, isOutput=False)
    kw1b_p = dp("kw1b", [128, 3, 3, 2, 2, 128], fp8d, isOutput=False)
    kw1c_p = dp("kw1c", [128, 2, 3, 2, 2, 128], fp8d, isOutput=False)
    zb_p = dp("zb", [128, 8], bf16, isOutput=False)
    keysA_p = dp("keysA", [128, 1032], bf16, isOutput=False)
    queriesA_p = dp("queriesA", [128, T1 + 2], bf16, isOutput=False)
    blobZ_p = dp("blobZ", [128, NZ], bf16, isOutput=False)
    blobM_p = dp("blobM", [128, NM], bf16, isOutput=False)
    blobF_p = dp("blobF", [128, NF], f32, isOutput=False)
    pb_p = dp("pb", [128, 8, T2], bf16, isOutput=False)
    out_p = dp("out", [T1, 2 * T2], bf16, isOutput=True)

    with _TC(nc) as tc, ExitStack() as ctx:
        const = ctx.enter_context(tc.tile_pool(name="const", bufs=1))
        work = ctx.enter_context(tc.tile_pool(name="work", bufs=4))
        small = ctx.enter_context(tc.tile_pool(name="small", bufs=8))

        # ---- warmup tiles (memset so sim sees initialized data)
        warm_w = const.tile([128, 8], bf16, tag="warmw")
        nc.vector.memset(warm_w, 0.0)
        warm_rhs = const.tile([128, 512], bf16, tag="warmrhs")
        nc.vector.memset(warm_rhs, 0.0)
        # fp8 conditioned keys for DoubleRow conv1; 272-col stride (16-mult)
        keys_f8 = const.tile([128, 4, 272], fp8d, tag="keysf8")
        nc.vector.memset(keys_f8[:, :, 0:1], 0.0)
        nc.vector.memset(keys_f8[:, :, 257:258], 0.0)

        # ---- input DMAs.  SP ring: fp8 weights; ACT ring: blobs + prior.
        # Critical set (gates compute start): wz, blobZ, blobA, blobF,
        # blobM, kw1a.  kw1b and pb are bandwidth-gated behind it so the
        # critical set gets the full ~360GB/s (total input is HBM-bound).
        # Q1 (SP ring) carries the strict critical chain in need-order:
        # wz (proj) -> keysA (conditioning) -> kw1a/b/c (conv1 blocks).
        # Q10 (ACT ring) carries the small/late tensors.
        wz_sb = const.tile([128, 8, 592], fp8, tag="wz")
        nc.sync.dma_start(out=wz_sb, in_=wz_p[:])
        keysA = const.tile([128, 1032], bf16, tag="keysA")
        nc.sync.dma_start(out=keysA, in_=keysA_p[:])
        kw1_sl = []
        for nm, p in (("kw1a", kw1a_p), ("kw1b", kw1b_p), ("kw1c", kw1c_p)):
            t = const.tile(list(p.shape), fp8d, tag=nm, name=nm)
            nc.sync.dma_start(out=t, in_=p[:])
            kw1_sl.append(t)
        zb_sb = const.tile([128, 8], bf16, tag="zb")
        nc.scalar.dma_start(out=zb_sb, in_=zb_p[:])
        blobF = const.tile([128, NF], f32, tag="blobF")
        nc.scalar.dma_start(out=blobF, in_=blobF_p[:])
        queriesA = const.tile([128, T1 + 2], bf16, tag="queriesA")
        nc.scalar.dma_start(out=queriesA, in_=queriesA_p[:])
        blobZ = const.tile([128, NZ], bf16, tag="blobZ")
        nc.scalar.dma_start(out=blobZ, in_=blobZ_p[:])
        blobM = const.tile([128, NM], bf16, tag="blobM")
        nc.scalar.dma_start(out=blobM, in_=blobM_p[:])
        pb_sb = const.tile([128, 8, T2], bf16, tag="pb")
        nc.scalar.dma_start(out=pb_sb, in_=pb_p[:])

        def kw1w(co, m, k):
            g = min(co // 3, 2)
            return kw1_sl[g][:, co - 3 * g, k, m]

        # persistent compute tiles
        keys1_bf = const.tile([128, 8, T2], bf16, tag="keys1")
        q1a_bf = const.tile([128, T1], bf16, tag="q1a")
        q1b_bf = const.tile([32, T1], bf16, tag="q1b")
        q2_bf = const.tile([MEL, T1], bf16, tag="q2bf")
        Qp = const.tile([MEL, T1], bf16, tag="Qp")
        kenc_sb = const.tile([ATT, T2], bf16, tag="kenc")
        sk_sb = const.tile([ATT, T2], bf16, tag="sk")
        k2neg_sb = const.tile([1, T2], bf16, tag="k2neg")
        proj_sb = const.tile([128, 5], f32, tag="proj_sb")

        # ---- warmup + proj on PE
        with tc.tile_pool(name="psA", bufs=1, space="PSUM") as psA:
            warm_ps = psA.tile([8, 512], f32, tag="warm")
            for _ in range(8):
                nc.tensor.matmul(
                    warm_ps, lhsT=warm_w, rhs=warm_rhs, start=True, stop=True
                )
            proj_ps = psA.tile([128, 5], f32, tag="proj")
            for cb in range(5):
                mw = 128 if cb < 4 else 80
                for jc in range(8):
                    nc.tensor.matmul(
                        proj_ps[0:mw, cb : cb + 1],
                        lhsT=wz_sb[:, jc, cb * 128 : cb * 128 + mw],
                        rhs=zb_sb[:, jc : jc + 1],
                        start=(jc == 0),
                        stop=(jc == 7),
                    )
                # the tiny proj matmuls don't register as PE activity; keep
                # the HAM clock gate open through proj + conditioning
                nc.tensor.matmul(
                    warm_ps, lhsT=warm_w, rhs=warm_rhs, start=True, stop=True
                )
            for _ in range(2):  # PE busy while DVE does proj-add + cond
                nc.tensor.matmul(
                    warm_ps, lhsT=warm_w, rhs=warm_rhs, start=True, stop=True
                )
            nc.vector.tensor_add(
                proj_sb[:, 0:4], proj_ps[:, 0:4], blobF[:, FBV : FBV + 4]
            )
            nc.vector.tensor_add(
                proj_sb[0:MEL, 4:5], proj_ps[0:MEL, 4:5],
                blobF[0:MEL, FBV + 4 : FBV + 5],
            )

        # ---- conditioning.  Queries in place (DVE, first: unblocks q1);
        # keys into keys_f8 (fp8 for DoubleRow): ci0/ci1 DVE, ci2/ci3 ACT
        # (Identity with per-partition bias) to halve the wall time.
        nc.vector.tensor_scalar_add(
            queriesA[0:MEL, 1 : 1 + T1],
            queriesA[0:MEL, 1 : 1 + T1],
            proj_sb[0:MEL, 4:5],
        )
        for ci in range(4):
            nc.vector.tensor_scalar_add(
                keys_f8[:, ci, 1:257],
                keysA[:, ci * 258 + 1 : ci * 258 + 257],
                proj_sb[:, ci : ci + 1],
            )

        # ---- conv chains (keys chain prioritized; q blocks fill gaps)
        with (
            tc.tile_pool(name="psB", bufs=1, space="PSUM") as psB,
            tc.tile_pool(name="psC", bufs=1, space="PSUM") as psC,
        ):
            def q1_block(t0):
                q1a_ps = psB.tile([128, 500], f32, tag="q1a")
                q1b_ps = psB.tile([32, 500], f32, tag="q1b")
                for k in range(3):
                    nc.tensor.matmul(
                        q1a_ps,
                        lhsT=blobZ[0:MEL, QW1 + k * 160 : QW1 + k * 160 + 128],
                        rhs=queriesA[0:MEL, t0 + k : t0 + k + 500],
                        start=(k == 0),
                        stop=(k == 2),
                    )
                for k in range(3):
                    nc.tensor.matmul(
                        q1b_ps,
                        lhsT=blobZ[0:MEL, QW1 + k * 160 + 128 : QW1 + k * 160 + 160],
                        rhs=queriesA[0:MEL, t0 + k : t0 + k + 500],
                        start=(k == 0),
                        stop=(k == 2),
                    )
                nc.vector.tensor_scalar(
                    q1a_bf[:, t0 : t0 + 500], q1a_ps,
                    blobF[:, FQB1A : FQB1A + 1], 0.0,
                    op0=mybir.AluOpType.add, op1=mybir.AluOpType.max,
                )
                nc.scalar.activation(
                    q1b_bf[:, t0 : t0 + 500], q1b_ps, AF.Relu,
                    bias=blobF[0:32, FQB1B : FQB1B + 1],
                )

            def q2_block(t0):
                q2_ps = psB.tile([MEL, 500], f32, tag="q2")
                nc.tensor.matmul(
                    q2_ps, lhsT=blobZ[:, QW2A : QW2A + MEL],
                    rhs=q1a_bf[:, t0 : t0 + 500], start=True, stop=False,
                )
                nc.tensor.matmul(
                    q2_ps, lhsT=blobZ[0:32, QW2B : QW2B + MEL],
                    rhs=q1b_bf[:, t0 : t0 + 500], start=False, stop=True,
                )
                nc.scalar.activation(
                    q2_bf[:, t0 : t0 + 500], q2_ps, AF.Relu,
                    bias=blobF[0:MEL, FQB2 : FQB2 + 1],
                )

            def q3_block(t0):
                q3_ps = psB.tile([MEL, 500], f32, tag="q2")
                nc.tensor.matmul(
                    q3_ps, lhsT=blobZ[0:MEL, QW3 : QW3 + MEL],
                    rhs=q2_bf[:, t0 : t0 + 500], start=True, stop=True,
                )
                nc.vector.tensor_scalar(
                    Qp[:, t0 : t0 + 500], q3_ps,
                    blobF[0:MEL, FQB3 : FQB3 + 1], 2.0 * TEMP,
                    op0=mybir.AluOpType.add, op1=mybir.AluOpType.mult,
                )

            k_ps = psC.tile([ATT, T2], f32, tag="kps", bufs=1)

            def conv1_block(co):
                c1_ps = psC.tile([128, T2], f32, tag="c1", bufs=4)
                n = 0
                for m in range(2):
                    for k in range(3):
                        v = keys_f8[:, 2 * m, k : k + T2]
                        rhs = bass.AP(
                            tensor=v.tensor, offset=v.offset,
                            ap=[list(v.ap[0]), [272, 2], [1, T2]],
                        )
                        nc.tensor.matmul(
                            c1_ps,
                            lhsT=kw1w(co, m, k),
                            rhs=rhs,
                            start=(n == 0),
                            stop=(n == 5),
                            perf_mode=mybir.MatmulPerfMode.DoubleRow,
                        )
                        n += 1
                if co % 2 == 0:
                    nc.scalar.activation(
                        keys1_bf[:, co, :], c1_ps, AF.Relu,
                        bias=blobF[:, FKB1 + co : FKB1 + co + 1],
                    )
                else:
                    nc.vector.tensor_scalar(
                        keys1_bf[:, co, :], c1_ps,
                        blobF[:, FKB1 + co : FKB1 + co + 1], 0.0,
                        op0=mybir.AluOpType.add, op1=mybir.AluOpType.max,
                    )
                if co >= 3:
                    conv2_block(co - 3)

            def conv2_block(co):
                nc.tensor.matmul(
                    k_ps,
                    lhsT=blobM[:, KW2 + co * MEL : KW2 + (co + 1) * MEL],
                    rhs=keys1_bf[:, co, :],
                    start=(co == 0),
                    stop=(co == 7),
                )

            q1_block(0)
            # keep the PE clock hot across the cond->conv1 data-wait window
            warm2_ps = psB.tile([8, 512], f32, tag="q2", name="warm2_ps")
            for _ in range(3):
                nc.tensor.matmul(
                    warm2_ps, lhsT=warm_w, rhs=warm_rhs, start=True, stop=True
                )
            conv1_block(0)
            conv1_block(1)
            q1_block(500)
            conv1_block(2)
            q2_block(0)
            conv1_block(3)
            q2_block(500)
            conv1_block(4)
            conv1_block(5)
            q3_block(0)
            conv1_block(6)
            conv1_block(7)
            conv2_block(5)
            conv2_block(6)
            conv2_block(7)
            q3_block(500)

            nc.vector.tensor_scalar_add(
                kenc_sb, k_ps, blobF[0:ATT, FKB2 : FKB2 + 1]
            )
            nc.scalar.activation(
                sk_sb, k_ps, AF.Square, bias=blobF[0:ATT, FKB2 : FKB2 + 1]
            )

        # ---- attention scores + double softmax, chunk-pipelined
        with (
            tc.tile_pool(name="psD", bufs=1, space="PSUM") as psD,
        ):
            k2_ps = psD.tile([1, T2], f32, tag="k2", bufs=1)
            nc.tensor.matmul(
                k2_ps, lhsT=blobZ[0:ATT, ONC : ONC + 1], rhs=sk_sb,
                start=True, stop=True,
            )
            nc.scalar.activation(k2neg_sb, k2_ps, AF.Copy, scale=-TEMP)

            rows_of = lambda c: 128 if c < 7 else T1 - 7 * 128
            a_tiles = {}
            for c in range(8):
                rows = rows_of(c)
                a_ps = psD.tile([128, T2], f32, tag="attn", bufs=7)
                nc.tensor.matmul(
                    a_ps[0:rows],
                    lhsT=Qp[:, c * 128 : c * 128 + rows],
                    rhs=kenc_sb,
                    start=True,
                    stop=False,
                )
                nc.tensor.matmul(
                    a_ps[0:rows],
                    lhsT=blobZ[0:1, ONR : ONR + rows],
                    rhs=k2neg_sb,
                    start=False,
                    stop=True,
                )
                a_tiles[c] = a_ps

            # epilogue, emission skewed so ACT stays dense:
            #   ACT queue: exp0 exp1 ln0 exp2 ln1 ... exp7 ln6 ln7
            e_t, s_t, r_t, ep_t, o_t = {}, {}, {}, {}, {}

            def ep_exp(c):
                rows = rows_of(c)
                e = work.tile([128, T2], bf16, tag="e", bufs=8, name="e")
                s = small.tile([128, 1], f32, tag="s", name="s")
                nc.scalar.activation(
                    e[0:rows], a_tiles[c][0:rows], AF.Exp, accum_out=s[0:rows]
                )
                e_t[c], s_t[c] = e, s

            def ep_mid(c):
                rows = rows_of(c)
                r = small.tile([128, 1], f32, tag="r", name="r")
                nc.vector.reciprocal(r[0:rows], s_t[c][0:rows])
                ep = work.tile([128, T2], bf16, tag="ep", bufs=8, name="ep")
                nc.gpsimd.tensor_mul(ep[0:rows], e_t[c][0:rows], pb_sb[0:rows, c, :])
                r_t[c], ep_t[c] = r, ep

            def ep_tail(c):
                rows = rows_of(c)
                osb = work.tile([128, 2 * T2], bf16, tag="osb", bufs=8, name="osb")
                nc.scalar.activation(
                    osb[0:rows, 0:T2], ep_t[c][0:rows], AF.Ln, scale=r_t[c][0:rows]
                )
                wm = work.tile([128, T2], bf16, tag="wm", bufs=8, name="wm")
                nc.vector.tensor_mul(
                    wm[0:rows], ep_t[c][0:rows], blobM[0:rows, PMK : PMK + T2]
                )
                s2 = small.tile([128, 1], f32, tag="s2", name="s2")
                nc.vector.reduce_sum(s2[0:rows], wm[0:rows], axis=AX.X)
                r2 = small.tile([128, 1], f32, tag="r2", name="r2")
                nc.vector.reciprocal(r2[0:rows], s2[0:rows])
                # per-partition scalar mult (keeps DVE 2x bf16 mode; a
                # stride-0 broadcast AP would force 1x)
                nc.vector.tensor_scalar_mul(
                    osb[0:rows, T2 : 2 * T2], wm[0:rows], r2[0:rows]
                )
                nc.sync.dma_start(
                    out=out_p[c * 128 : c * 128 + rows, :], in_=osb[0:rows, :]
                )

            ep_exp(0)
            ep_exp(1)
            for c in range(8):
                if c < 6:
                    ep_exp(c + 2)
                ep_mid(c)
                ep_tail(c)

    if split_waits:
        _split_excess_waits(nc)
    return nc


_NC_CACHE = {}


def _get_nc():
    if "nc" not in _NC_CACHE:
        _NC_CACHE["nc"] = _build_nc()
    return _NC_CACHE["nc"]


def _prep_in_maps(inputs):
    q = np.asarray(inputs["queries"], F32)
    k = np.asarray(inputs["keys"], F32)
    mask = np.asarray(inputs["mask"])
    prior = np.asarray(inputs["attn_prior"], F32)
    spk = np.asarray(inputs["speaker_embed"], F32)
    emo = np.asarray(inputs["emotion_embed"], F32)

    kw1 = np.asarray(inputs["kw1"], F32)
    kb1 = np.asarray(inputs["kb1"], F32)
    kw2 = np.asarray(inputs["kw2"], F32)
    kb2 = np.asarray(inputs["kb2"], F32)
    qw1 = np.asarray(inputs["qw1"], F32)
    qb1 = np.asarray(inputs["qb1"], F32)
    qw2 = np.asarray(inputs["qw2"], F32)
    qb2 = np.asarray(inputs["qb2"], F32)
    qw3 = np.asarray(inputs["qw3"], F32)
    qb3 = np.asarray(inputs["qb3"], F32)
    spk_kw = np.asarray(inputs["spk_kw"], F32)
    spk_kb = np.asarray(inputs["spk_kb"], F32)
    spk_qw = np.asarray(inputs["spk_qw"], F32)
    spk_qb = np.asarray(inputs["spk_qb"], F32)
    emo_kw = np.asarray(inputs["emo_kw"], F32)
    emo_kb = np.asarray(inputs["emo_kb"], F32)
    emo_qw = np.asarray(inputs["emo_qw"], F32)
    emo_qb = np.asarray(inputs["emo_qb"], F32)

    # wz: concat speaker/emotion proj weights, x64, fp8  [128, 8, 592]
    wcat = np.concatenate(
        [
            np.concatenate([spk_kw, emo_kw], axis=1),
            np.concatenate([spk_qw, emo_qw], axis=1),
        ],
        axis=0,
    )  # (592, 1024)
    wz = np.ascontiguousarray(
        (wcat.T.reshape(8, 128, 592).transpose(1, 0, 2)) * WSW
    ).astype(FP8)

    # kw1 x64 fp8, (j, co, ci, k, c); split over co
    # pairs layout [j, co_hi, k, m, sub, c] (sub = paired ci blocks)
    kw1_t = (
        kw1.reshape(8, 128, 2, 2, 128, 3).transpose(4, 0, 5, 2, 3, 1) * WS
    ).astype(FP8D)
    kw1a = np.ascontiguousarray(kw1_t[:, 0:3])
    kw1b = np.ascontiguousarray(kw1_t[:, 3:6])
    kw1c = np.ascontiguousarray(kw1_t[:, 6:8])

    k_r = k.reshape(B, 4, 128, T2)
    # blobZ (bf16, early) / blobM (bf16, late)
    blobZ = np.zeros((128, NZ), BF16)
    blobM = np.zeros((128, NM), BF16)
    blobZ[:, ONC] = 1.0
    blobZ[:, ONR : ONR + 128] = 1.0
    for kk in range(3):
        # lhsT layout: [j(80 partitions), c(160 cols)] = qw1[c, j, kk].T
        blobZ[0:MEL, QW1 + kk * 160 : QW1 + (kk + 1) * 160] = np.ascontiguousarray(
            qw1[:, :, kk]
        ).T.astype(BF16)
    qw2t = qw2[:, :, 0].T  # (160, 80): [in-ch j, out-ch]
    blobZ[:, QW2A : QW2A + MEL] = qw2t[0:128].astype(BF16)
    blobZ[0:32, QW2B : QW2B + MEL] = qw2t[128:160].astype(BF16)
    blobZ[0:MEL, QW3 : QW3 + MEL] = qw3[:, :, 0].T.astype(BF16)
    pmask_row = (1.0 - mask[:, :, 0].astype(F32))  # (B, T2)
    kw2_t = (kw2[:, :, 0].T.reshape(8, 128, ATT).transpose(1, 0, 2) / WS).astype(BF16)
    blobM[:, KW2 : KW2 + 640] = kw2_t.reshape(128, 640)

    # blobF (f32)
    blobF = np.zeros((128, NF), F32)
    bcat = np.zeros(640, F32)
    bcat[0:TXT] = spk_kb + emo_kb
    bcat[TXT : TXT + MEL] = spk_qb + emo_qb
    blobF[:, FBV : FBV + 5] = bcat.reshape(5, 128).T
    blobF[:, FKB1 : FKB1 + 8] = kb1.reshape(8, 128).T * WS
    blobF[0:ATT, FKB2] = kb2
    blobF[:, FQB1A] = qb1[0:128]
    blobF[0:32, FQB1B] = qb1[128:160]
    blobF[0:MEL, FQB2] = qb2
    blobF[0:MEL, FQB3] = qb3

    z_all = np.concatenate([spk, emo], axis=1)  # (B, 1024)

    in_maps = []
    for b in range(B):
        m = {
            "wz": wz,
            "kw1a": kw1a,
            "kw1b": kw1b,
            "kw1c": kw1c,
        }
        kA = np.zeros((128, 1032), BF16)
        for ci in range(4):
            kA[:, ci * 258 + 1 : ci * 258 + 257] = k_r[b, ci].astype(BF16)
        m["keysA"] = kA
        qA = np.zeros((128, T1 + 2), BF16)
        qA[0:MEL, 1 : 1 + T1] = q[b].astype(BF16)
        m["queriesA"] = qA
        m["zb"] = np.ascontiguousarray(
            (z_all[b].reshape(8, 128).T / WSW)
        ).astype(BF16)
        m["blobZ"] = blobZ
        bM = blobM.copy()
        bM[:, PMK : PMK + T2] = np.broadcast_to(
            pmask_row[b].astype(BF16), (128, T2)
        )
        m["blobM"] = bM
        m["blobF"] = blobF
        pbt = np.zeros((128, 8, T2), BF16)
        pr = prior[b] + 1e-8  # (1000, 256)
        pr_pad = np.zeros((1024, T2), F32)
        pr_pad[0:T1] = pr
        pbt[:, :, :] = pr_pad.reshape(8, 128, T2).transpose(1, 0, 2).astype(BF16)
        m["pb"] = pbt
        in_maps.append(m)
    return in_maps


def _unpack(results):
    attn = np.stack(
        [
            np.asarray(results[i]["out"][:, T2 : 2 * T2], F32)
            for i in range(N_CORES)
        ]
    )
    logp = np.stack(
        [np.asarray(results[i]["out"][:, 0:T2], F32) for i in range(N_CORES)]
    )
    return attn[:, None], logp[:, None]


def kernel(**inputs):
    from concourse.bass_utils import run_bass_kernel_spmd

    nc = _get_nc()
    in_maps = _prep_in_maps(inputs)
    res = run_bass_kernel_spmd(nc, in_maps, core_ids=list(range(N_CORES)))
    return _unpack(res.results)
